# revision 55
# baseline (speedup 1.0000x reference)
"""Masked attention kernel for Trainium2, data-parallel over batch on 8 NeuronCores.

Problem (per reference):
    query (128, 512) f32, key/value (1024, 128, 512) f32, mask (128, 1, 1024) i32
    energy = einsum('bh,tbh->bt'); attn = softmax(energy)
    masked = mask*attn / sum(mask*attn); context = einsum('bt,tbh->bh')
    returns (context (128,512), masked_attention (128,1024))

Key algebraic simplification: the unmasked softmax normalizer cancels:
    masked = m*exp(e-max) / sum(m*exp(e-max))

Per-core structure (B_loc=16, T=1024, H=512; t on partitions, 8 t-tiles):
    - q broadcast on-chip: 2 KB row DMAs + PE outer products (ones^T @ q_row)
    - K and V streamed as (128 t, 8 half-batch, 512 h) tiles: 16 KB contiguous
      bursts at 32 KB stride, 2 MB per DMA
    - energy: fused DVE affine_mul_reduce (K*q_bcast, sum over h) one pass
      -> E_all (128 t-part, tt*16+b columns)
    - PE transposes E to row layout (16 b-part, 1024 t); softmax row-wise:
      reduce_max(negate) -> ACT Exp(bias=-max) -> mask mult + sum -> reciprocal
    - attn rows scaled by 1/Z, DMA'd out; attn transposed back to columns
    - context: all 128 PE matmuls (float32r) accumulate into ONE (16,512)
      psum tile; lhsT is column-masked (only column b nonzero) so each row
      accumulates exactly its own batch. One copy + one 32 KB output DMA.
"""

import numpy as np

B, T, H = 128, 1024, 512
NCORES = 8
BL = B // NCORES  # 16 batches per core
NT = T // 128     # 8 t-tiles
HB = BL // 2      # half-batch chunk (K/V tiles)

_cache = {}


def _build_nc(debug=False):
    from contextlib import ExitStack

    import concourse.bacc as bacc
    import concourse.bass as bass
    import concourse.mybir as mybir
    import concourse.tile as tile
    from concourse import masks

    f32 = mybir.dt.float32
    f32r = mybir.dt.float32r
    i32 = mybir.dt.int32
    Alu = mybir.AluOpType
    Act = mybir.ActivationFunctionType

    nc = bacc.Bacc("TRN2", target_bir_lowering=False, debug=debug)

    q_d = nc.dram_tensor("query", [BL, H], f32, kind="ExternalInput")
    k_d = nc.dram_tensor("key", [T, BL, H], f32, kind="ExternalInput")
    v_d = nc.dram_tensor("value", [T, BL, H], f32r, kind="ExternalInput")
    m_d = nc.dram_tensor("mask", [BL, 1, T], i32, kind="ExternalInput")
    ctx_d = nc.dram_tensor("out_ctx", [BL, H], f32, kind="ExternalOutput")
    attn_d = nc.dram_tensor("out_attn", [BL, T], f32, kind="ExternalOutput")

    # (T, BL, H) -> (NT, 2, 128, HB, H): per (t-tile, half-batch) tiles.
    # Each [tt, hf] slice = 128 partitions x 16 KB contiguous, 32 KB stride.
    k_r = k_d.ap().rearrange("(tt p) (hf b) h -> tt hf p b h", p=128, b=HB)
    # V in quarter-batch chunks (1 MB DMAs): faster slot turnover around
    # the softmax boundary
    QB = BL // 4
    v_r = v_d.ap().rearrange("(tt p) (qf b) h -> tt qf p b h", p=128, b=QB)

    with tile.TileContext(nc) as tc, ExitStack() as ctx:
        const = ctx.enter_context(tc.tile_pool(name="const", bufs=1))
        once = ctx.enter_context(tc.tile_pool(name="once", bufs=1))
        kpool = ctx.enter_context(tc.tile_pool(name="kpool", bufs=3))
        vpool = ctx.enter_context(tc.tile_pool(name="vpool", bufs=8))
        prodp = ctx.enter_context(tc.tile_pool(name="prodp", bufs=2))
        psum_e = ctx.enter_context(
            tc.tile_pool(name="psum_e", bufs=1, space=bass.MemorySpace.PSUM)
        )
        psum_s = ctx.enter_context(
            tc.tile_pool(name="psum_s", bufs=2, space=bass.MemorySpace.PSUM)
        )
        lhsp = ctx.enter_context(tc.tile_pool(name="lhsp", bufs=16))
        psum_w = ctx.enter_context(
            tc.tile_pool(name="psum_w", bufs=1, space=bass.MemorySpace.PSUM)
        )
        psum_c = ctx.enter_context(
            tc.tile_pool(name="psum_c", bufs=1, space=bass.MemorySpace.PSUM)
        )

        identity = const.tile([128, 128], f32)
        masks.make_identity(nc, identity[:])

        mask_i = once.tile([BL, T], i32)
        nc.sync.dma_start(mask_i[:], m_d.ap().rearrange("b o t -> b (o t)"))
        mask_f = const.tile([BL, T], f32)
        nc.vector.tensor_copy(mask_f[:], mask_i[:])

        # one-hot column masks for the context accumulation:
        # colmask[:, b*BL + j] = 1.0 iff j == b
        colmask = const.tile([128, BL * BL], f32)
        nc.gpsimd.memset(colmask[:], 0.0)
        for b in range(BL):
            nc.vector.memset(colmask[:, b * BL + b : b * BL + b + 1], 1.0)

        # broadcast each query row to all 128 partitions via 0-stride DRAM
        # reads. Costs 4 MB of redundant HBM traffic, but lands first on the
        # DMA queue with no compute dependencies: a PE-based broadcast stalls
        # the whole K stream ~30 us behind cold fp32 matmuls at startup.
        # issued from ACT's queue row: these broadcast reads are slow per
        # byte and must not delay the K stream on the sync queue
        qb = const.tile([128, BL, H], f32)
        for b in range(BL):
            nc.scalar.dma_start(
                qb[:, b, :], q_d.ap()[b : b + 1, :].to_broadcast((128, H))
            )

        # ---- energy: E_all[p, tt*BL+b] = sum_h K[tt*128+p, b, h] * q[b, h]
        # tt-outer so each t-tile's transpose + partial row-max pipeline
        # behind the K stream instead of serializing at the end
        E_all = const.tile([128, NT * BL], f32)
        erow = psum_e.tile([BL, T], f32)
        pmax = const.tile([BL, NT], f32)
        k_gate = None
        for tt in range(NT):
            for hf in range(2):
                kt = kpool.tile([128, HB, H], f32, tag="kt")
                kd = nc.sync.dma_start(kt[:], k_r[tt, hf])
                if tt * 2 + hf == 9:
                    k_gate = kd

                # balance the reduction between DVE (fused mult+reduce) and
                # ACT (reduce of a DVE-computed product); GpSimd is useless
                # here — it shares (and exclusively locks) DVE's SBUF port
                for j in range(HB):
                    b = hf * HB + j
                    col = E_all[:, tt * BL + b : tt * BL + b + 1]
                    pr = prodp.tile([128, H], f32, tag="pr")
                    if j < 3:
                        nc.vector.affine_mul_reduce(
                            out=pr[:],
                            accum_out=col,
                            in0=kt[:, j, :],
                            in1=qb[:, b, :],
                            scale=1.0,
                            bias=0.0,
                        )
                    else:
                        nc.vector.tensor_tensor(
                            out=pr[:], in0=kt[:, j, :], in1=qb[:, b, :], op=Alu.mult
                        )
                        pr2 = psum_s.tile([128, H], f32)
                        nc.scalar.activation(
                            pr2[:], pr[:], Act.Identity, accum_out=col
                        )
            # transpose this t-tile to row layout and take its partial max
            nc.tensor.transpose(
                erow[:, tt * 128 : (tt + 1) * 128],
                E_all[:, tt * BL : (tt + 1) * BL],
                identity[:],
            )
            nc.vector.tensor_reduce(
                pmax[:, tt : tt + 1],
                erow[:, tt * 128 : (tt + 1) * 128],
                axis=mybir.AxisListType.X,
                op=Alu.max,
            )

        # ---- softmax (row-wise over free dim)
        negmax = const.tile([BL, 1], f32)
        nc.vector.tensor_reduce(
            negmax[:], pmax[:], axis=mybir.AxisListType.X, op=Alu.max, negate=True
        )
        xrow = once.tile([BL, T], f32)
        nc.scalar.activation(xrow[:], erow[:], Act.Exp, bias=negmax[:], scale=1.0)
        # fused: wrow = xrow * mask, zsum = row-sum(wrow)
        wrow = const.tile([BL, T], f32)
        zsum = const.tile([BL, 1], f32)
        nc.vector.affine_mul_reduce(
            out=wrow[:], accum_out=zsum[:], in0=xrow[:], in1=mask_f[:],
            scale=1.0, bias=0.0,
        )
        rz = const.tile([BL, 1], f32)
        nc.vector.reciprocal(rz[:], zsum[:])

        # transpose UNNORMALIZED weights — the 1/Z scale is applied to the
        # attn output (off the critical path) and folded into the context
        # epilogue copy, so the matmuls start as soon as wrow exists
        wcol_ps = psum_w.tile([128, NT * BL], f32)
        for tt in range(NT):
            nc.tensor.transpose(
                wcol_ps[:, tt * BL : (tt + 1) * BL],
                wrow[:, tt * 128 : (tt + 1) * 128],
                identity[:BL, :BL],
            )
        wcol = const.tile([128, NT * BL], f32)
        nc.scalar.copy(wcol[:], wcol_ps[:])

        attn = const.tile([BL, T], f32)
        nc.vector.tensor_scalar_mul(attn[:], wrow[:], rz[:])
        # output DMAs go on the ACT queue: the sync queue is FIFO per engine,
        # and a compute-dependent DMA there would head-of-line block V loads
        nc.scalar.dma_start(attn_d.ap(), attn[:])

        # ---- context: ctx[b, h] = sum_t attn[b, t] * V[t, b, h]
        # All 128 float32r matmuls accumulate into one (16,512) psum tile.
        # lhsT for (tt, b) is wcol's tt block masked to column b only, so
        # psum row b accumulates exactly batch b's contributions.
        from concourse.tile import add_dep_helper

        cps = psum_c.tile([BL, H], f32)
        nmm = NT * BL
        i = 0
        for tt in range(NT):
            for qf in range(4):
                # V loads issue from GpSimd (SWDGE): a separate DMA queue row,
                # so V prefetch fills the gaps whenever the K stream is
                # slot-blocked, without FIFO coupling to the sync queue.
                # The first chunks are gated on K tile #10 so the prefetch
                # lands at the softmax boundary instead of starving early K.
                vt = vpool.tile([128, QB, H], f32r, tag="vt")
                vd = nc.gpsimd.dma_start(vt[:], v_r[tt, qf])
                if tt * 4 + qf < 8 and k_gate is not None:
                    add_dep_helper(
                        vd.ins, k_gate.ins, sync=True,
                        reason="gate V prefetch behind late K stream",
                    )
                for j in range(QB):
                    b = qf * QB + j
                    lhsT = lhsp.tile([128, BL], f32r, tag="lhsT")
                    nc.vector.tensor_tensor(
                        out=lhsT[:],
                        in0=wcol[:, tt * BL : (tt + 1) * BL],
                        in1=colmask[:, b * BL : (b + 1) * BL],
                        op=Alu.mult,
                    )
                    nc.tensor.matmul(
                        cps[:],
                        lhsT[:],
                        vt[:, j, :],
                        start=(i == 0),
                        stop=(i == nmm - 1),
                    )
                    i += 1
        # epilogue: one copy with the 1/Z row scale folded in, then DMA out
        ctx_sb = const.tile([BL, H], f32)
        nc.scalar.activation(ctx_sb[:], cps[:], Act.Copy, scale=rz[:])
        nc.scalar.dma_start(ctx_d.ap(), ctx_sb[:])

    nc.compile()
    return nc


def _get_nc():
    if "nc" not in _cache:
        _cache["nc"] = _build_nc(debug=False)
    return _cache["nc"]


def _shard_inputs(query, key, value, mask):
    in_maps = []
    for i in range(NCORES):
        s = slice(i * BL, (i + 1) * BL)
        in_maps.append(
            {
                "query": np.ascontiguousarray(query[s]),
                "key": np.ascontiguousarray(key[:, s]),
                "value": np.ascontiguousarray(value[:, s]),
                "mask": np.ascontiguousarray(mask[s]),
            }
        )
    return in_maps


def run_sharded(query, key, value, mask, trace=False, **kw):
    from concourse.bass_utils import run_bass_kernel_spmd

    nc = _get_nc()
    in_maps = _shard_inputs(query, key, value, mask)
    res = run_bass_kernel_spmd(
        nc, in_maps, core_ids=list(range(NCORES)), trace=trace, **kw
    )
    context = np.concatenate([res.results[i]["out_ctx"] for i in range(NCORES)], axis=0)
    attn = np.concatenate([res.results[i]["out_attn"] for i in range(NCORES)], axis=0)
    return (context, attn), res


def kernel(query, key, value, mask):
    query = np.asarray(query, dtype=np.float32)
    key = np.asarray(key, dtype=np.float32)
    value = np.asarray(value, dtype=np.float32)
    mask = np.asarray(mask, dtype=np.int32)
    (context, attn), _ = run_sharded(query, key, value, mask, trace=False)
    return (context, attn)


# revision 58
# speedup vs baseline: 1.0570x; 1.0570x over previous
"""Masked attention kernel for Trainium2, data-parallel over batch on 8 NeuronCores.

Problem (per reference):
    query (128, 512) f32, key/value (1024, 128, 512) f32, mask (128, 1, 1024) i32
    energy = einsum('bh,tbh->bt'); attn = softmax(energy)
    masked = mask*attn / sum(mask*attn); context = einsum('bt,tbh->bh')
    returns (context (128,512), masked_attention (128,1024))

Key algebraic simplification: the unmasked softmax normalizer cancels:
    masked = m*exp(e-max) / sum(m*exp(e-max))

Per-core structure (B_loc=16, T=1024, H=512; t on partitions, 8 t-tiles):
    - q broadcast on-chip: 2 KB row DMAs + PE outer products (ones^T @ q_row)
    - K and V streamed as (128 t, 8 half-batch, 512 h) tiles: 16 KB contiguous
      bursts at 32 KB stride, 2 MB per DMA
    - energy: fused DVE affine_mul_reduce (K*q_bcast, sum over h) one pass
      -> E_all (128 t-part, tt*16+b columns)
    - PE transposes E to row layout (16 b-part, 1024 t); softmax row-wise:
      reduce_max(negate) -> ACT Exp(bias=-max) -> mask mult + sum -> reciprocal
    - attn rows scaled by 1/Z, DMA'd out; attn transposed back to columns
    - context: all 128 PE matmuls (float32r) accumulate into ONE (16,512)
      psum tile; lhsT is column-masked (only column b nonzero) so each row
      accumulates exactly its own batch. One copy + one 32 KB output DMA.
"""

import numpy as np

B, T, H = 128, 1024, 512
NCORES = 8
BL = B // NCORES  # 16 batches per core
NT = T // 128     # 8 t-tiles
HB = BL // 2      # half-batch chunk (K/V tiles)

_cache = {}


def _build_nc(debug=False):
    from contextlib import ExitStack

    import concourse.bacc as bacc
    import concourse.bass as bass
    import concourse.mybir as mybir
    import concourse.tile as tile
    from concourse import masks

    f32 = mybir.dt.float32
    f32r = mybir.dt.float32r
    i32 = mybir.dt.int32
    Alu = mybir.AluOpType
    Act = mybir.ActivationFunctionType

    nc = bacc.Bacc("TRN2", target_bir_lowering=False, debug=debug)

    q_d = nc.dram_tensor("query", [BL, H], f32, kind="ExternalInput")
    k_d = nc.dram_tensor("key", [T, BL, H], f32, kind="ExternalInput")
    v_d = nc.dram_tensor("value", [T, BL, H], f32r, kind="ExternalInput")
    m_d = nc.dram_tensor("mask", [BL, 1, T], i32, kind="ExternalInput")
    ctx_d = nc.dram_tensor("out_ctx", [BL, H], f32, kind="ExternalOutput")
    attn_d = nc.dram_tensor("out_attn", [BL, T], f32, kind="ExternalOutput")

    # (T, BL, H) -> (NT, 2, 128, HB, H): per (t-tile, half-batch) tiles.
    # Each [tt, hf] slice = 128 partitions x 16 KB contiguous, 32 KB stride.
    k_r = k_d.ap().rearrange("(tt p) (hf b) h -> tt hf p b h", p=128, b=HB)
    # V in quarter-batch chunks (1 MB DMAs): faster slot turnover around
    # the softmax boundary
    QB = BL // 4
    v_r = v_d.ap().rearrange("(tt p) (qf b) h -> tt qf p b h", p=128, b=QB)

    with tile.TileContext(nc) as tc, ExitStack() as ctx:
        const = ctx.enter_context(tc.tile_pool(name="const", bufs=1))
        once = ctx.enter_context(tc.tile_pool(name="once", bufs=1))
        kpool = ctx.enter_context(tc.tile_pool(name="kpool", bufs=3))
        vpool = ctx.enter_context(tc.tile_pool(name="vpool", bufs=8))
        prodp = ctx.enter_context(tc.tile_pool(name="prodp", bufs=2))
        psum_e = ctx.enter_context(
            tc.tile_pool(name="psum_e", bufs=1, space=bass.MemorySpace.PSUM)
        )
        psum_s = ctx.enter_context(
            tc.tile_pool(name="psum_s", bufs=2, space=bass.MemorySpace.PSUM)
        )
        lhsp = ctx.enter_context(tc.tile_pool(name="lhsp", bufs=16))
        psum_w = ctx.enter_context(
            tc.tile_pool(name="psum_w", bufs=1, space=bass.MemorySpace.PSUM)
        )
        psum_c = ctx.enter_context(
            tc.tile_pool(name="psum_c", bufs=1, space=bass.MemorySpace.PSUM)
        )

        identity = const.tile([128, 128], f32)
        masks.make_identity(nc, identity[:])

        # two half-batch mask tiles, both at partition base 0 (engines
        # cannot address partition bases outside {0,32,64,96})
        m_rows = m_d.ap().rearrange("b o t -> b (o t)")
        mask_i = [once.tile([HB, T], i32, name=f"mask_i{k}", tag=f"mask_i{k}") for k in range(2)]
        for hf in range(2):
            nc.sync.dma_start(mask_i[hf][:], m_rows[hf * HB : (hf + 1) * HB, :])

        # one-hot column masks for the context accumulation (8-wide)
        colmask8 = const.tile([128, HB * HB], f32)
        nc.gpsimd.memset(colmask8[:], 0.0)
        for j in range(HB):
            nc.vector.memset(colmask8[:, j * HB + j : j * HB + j + 1], 1.0)

        # broadcast each query row to all 128 partitions via 0-stride DRAM
        # reads on ACT's queue row (slow per byte, but zero compute deps and
        # off the K stream's queue)
        qb = const.tile([128, BL, H], f32)
        for b in range(BL):
            nc.scalar.dma_start(
                qb[:, b, :], q_d.ap()[b : b + 1, :].to_broadcast((128, H))
            )

        from concourse.tile import add_dep_helper

        # ---- half-batch pipeline: softmax rows are independent, so batches
        # 0-7 run their softmax + context matmuls while batches 8-15 are
        # still streaming K — the softmax bubble overlaps the K stream.
        E_all = const.tile([128, NT * BL], f32)
        mask_f = const.tile([HB, T], f32)
        xrow = once.tile([HB, T], f32)
        wrow = const.tile([HB, T], f32)
        attn_sh = const.tile([HB, T], f32)
        zsum = const.tile([HB, 1], f32)
        ctx_sb = const.tile([HB, H], f32)
        wcol_ps = psum_w.tile([128, NT, 2, HB], f32)
        wcol = const.tile([128, NT, 2, HB], f32)
        k_gate = None
        for hf in range(2):
            # ---- energy for this half: E_all[p, tt*BL+hf*HB+j]
            erow = psum_e.tile([HB, T], f32)
            pmax = const.tile([HB, NT], f32, tag=f"pmax{hf}")
            for tt in range(NT):
                kt = kpool.tile([128, HB, H], f32, tag="kt")
                kd = nc.sync.dma_start(kt[:], k_r[tt, hf])
                if hf == 0 and tt == 5:
                    k_gate = kd
                for j in range(HB):
                    b = hf * HB + j
                    col = E_all[:, tt * BL + b : tt * BL + b + 1]
                    pr = prodp.tile([128, H], f32, tag="pr")
                    if j < 3:
                        nc.vector.affine_mul_reduce(
                            out=pr[:], accum_out=col,
                            in0=kt[:, j, :], in1=qb[:, b, :],
                            scale=1.0, bias=0.0,
                        )
                    else:
                        nc.vector.tensor_tensor(
                            out=pr[:], in0=kt[:, j, :], in1=qb[:, b, :],
                            op=Alu.mult,
                        )
                        pr2 = psum_s.tile([128, H], f32)
                        nc.scalar.activation(
                            pr2[:], pr[:], Act.Identity, accum_out=col
                        )
                nc.tensor.transpose(
                    erow[:, tt * 128 : (tt + 1) * 128],
                    E_all[:, tt * BL + hf * HB : tt * BL + hf * HB + HB],
                    identity[:],
                )
                nc.vector.tensor_reduce(
                    pmax[:, tt : tt + 1],
                    erow[:, tt * 128 : (tt + 1) * 128],
                    axis=mybir.AxisListType.X, op=Alu.max,
                )

            # ---- softmax for this half (rows at base 0)
            negmax = const.tile([HB, 1], f32, tag=f"negmax{hf}")
            nc.vector.tensor_reduce(
                negmax[:], pmax[:], axis=mybir.AxisListType.X,
                op=Alu.max, negate=True,
            )
            nc.scalar.activation(
                xrow[:], erow[:], Act.Exp, bias=negmax[:], scale=1.0
            )
            nc.vector.tensor_copy(mask_f[:], mask_i[hf][:])
            nc.vector.affine_mul_reduce(
                out=wrow[:], accum_out=zsum[:], in0=xrow[:], in1=mask_f[:],
                scale=1.0, bias=0.0,
            )
            rz = const.tile([HB, 1], f32, tag=f"rz{hf}")
            nc.vector.reciprocal(rz[:], zsum[:])
            # transpose UNNORMALIZED weights; 1/Z is folded into the
            # epilogue copy and the attn output scale (off critical path)
            for tt in range(NT):
                nc.tensor.transpose(
                    wcol_ps[:, tt, hf, :],
                    wrow[:, tt * 128 : (tt + 1) * 128],
                    identity[:HB, :HB],
                )
            nc.scalar.copy(wcol[:, :, hf, :], wcol_ps[:, :, hf, :])
            nc.vector.tensor_scalar_mul(attn_sh[:], wrow[:], rz[:])
            nc.scalar.dma_start(
                attn_d.ap()[hf * HB : (hf + 1) * HB, :], attn_sh[:]
            )

            # ---- context for this half: 64 float32r matmuls into one
            # (8,512) psum tile; V chunks on the SWDGE queue
            cps = psum_c.tile([HB, H], f32)
            i = 0
            for tt in range(NT):
                for qq in range(2):
                    qf = 2 * hf + qq
                    vt = vpool.tile([128, QB, H], f32r, tag="vt")
                    vd = nc.gpsimd.dma_start(vt[:], v_r[tt, qf])
                    if hf == 0 and tt == 0 and k_gate is not None:
                        add_dep_helper(
                            vd.ins, k_gate.ins, sync=True,
                            reason="gate V prefetch behind late K-half0",
                        )
                    for j in range(QB):
                        bw = qq * QB + j
                        lhsT = lhsp.tile([128, HB], f32r, tag="lhsT")
                        nc.vector.tensor_tensor(
                            out=lhsT[:],
                            in0=wcol[:, tt, hf, :],
                            in1=colmask8[:, bw * HB : (bw + 1) * HB],
                            op=Alu.mult,
                        )
                        nc.tensor.matmul(
                            cps[:], lhsT[:], vt[:, j, :],
                            start=(i == 0), stop=(i == 2 * NT * QB - 1),
                        )
                        i += 1
            nc.scalar.activation(ctx_sb[:], cps[:], Act.Copy, scale=rz[:])
            nc.scalar.dma_start(
                ctx_d.ap()[hf * HB : (hf + 1) * HB, :], ctx_sb[:]
            )

    nc.compile()
    return nc


def _get_nc():
    if "nc" not in _cache:
        _cache["nc"] = _build_nc(debug=False)
    return _cache["nc"]


def _shard_inputs(query, key, value, mask):
    in_maps = []
    for i in range(NCORES):
        s = slice(i * BL, (i + 1) * BL)
        in_maps.append(
            {
                "query": np.ascontiguousarray(query[s]),
                "key": np.ascontiguousarray(key[:, s]),
                "value": np.ascontiguousarray(value[:, s]),
                "mask": np.ascontiguousarray(mask[s]),
            }
        )
    return in_maps


def run_sharded(query, key, value, mask, trace=False, **kw):
    from concourse.bass_utils import run_bass_kernel_spmd

    nc = _get_nc()
    in_maps = _shard_inputs(query, key, value, mask)
    res = run_bass_kernel_spmd(
        nc, in_maps, core_ids=list(range(NCORES)), trace=trace, **kw
    )
    context = np.concatenate([res.results[i]["out_ctx"] for i in range(NCORES)], axis=0)
    attn = np.concatenate([res.results[i]["out_attn"] for i in range(NCORES)], axis=0)
    return (context, attn), res


def kernel(query, key, value, mask):
    query = np.asarray(query, dtype=np.float32)
    key = np.asarray(key, dtype=np.float32)
    value = np.asarray(value, dtype=np.float32)
    mask = np.asarray(mask, dtype=np.int32)
    (context, attn), _ = run_sharded(query, key, value, mask, trace=False)
    return (context, attn)
